# revision 4
# baseline (speedup 1.0000x reference)
"""Trainium2 Bass kernel for nn_GAT_39427799777563 (GAT message passing).

Math (per item row n, K=32 neighbors, D=100 dims):
    We   = entity_embs * w_r                  # [K, D] elementwise
    e_k  = sum_d We[k, d]                     # neighbor logits
    a_k  = softmax_k(leaky_relu(e_k)) masked by adj
    h'   = sum_k a_k * We[k, :]               # weighted neighbor sum
    x    = h' @ W_out.T + b_out + item_embs

v2 design (vs the fp32 v1 at ~307us):
  * fp16 everywhere on the wire: ent/wr are loaded as one interleaved
    fp16 buffer (halves HBM traffic, the roofline term). fp16 (not bf16)
    because exp() amplifies e-sum rounding ~10x: bf16 inputs alone give
    1.7e-2 absmax-rel (gate 2e-2); fp16 gives 7.3e-3 (simulated).
  * k-innermost layout [row, j, d, k]: every elementwise op and tree-add
    has a packed 2-byte innermost AP dim, which turns on the DVE 2x mode
    (tensor_tensor 2x_1p). Crucially the attention-broadcast multiply
    q = We * a (broadcast over d) keeps k innermost-contiguous, so it
    runs 2x too - impossible in d-innermost layout (stride-0 innermost).
  * reductions as fp16 tree-adds (tensor_reduce never gets the 2x mode;
    tensor_tensor does): e-sum over d and h'-sum over k each cost ~half
    a strided reduce, with fp32 final level.
  * attention normalized BEFORE weighting (a = p/sum(p) in [0,1], fp16-
    safe; raw exp(e) ~ 1e17 is not), so the matmul epilogue is a plain
    residual add.
  * adj mask folded into the host packing: masked/padding slots get a
    poison pair ent[d0] = -244, wr[d0] = 244 (product -59536, exp -> 0
    exactly in fp32), so no adj tensor is loaded and no mask multiply.
  * engine balance per 256-row pair (DMA floor ~4.8us): DVE does the two
    big 2x multiplies + most tree levels (~5us); ACT takes the e-sums of
    the last few k's via activation(Copy, accum_out) plus exp and the
    PSUM->SBUF copies (~4.5us); GPSIMD takes the h'-tree first level and
    the residual epilogue (~3.8us); PE does transpose + the 100x100
    linear in fp16.

Sparsity packing as v1: active k's packed front per row, rows sorted by
count, 256-row pairs striped across the 8 SPMD cores, per-pair-slot K =
max over its 8 cores. Rows un-permuted on host after the gather.
"""

from contextlib import ExitStack

import numpy as np

import concourse.bass as bass
import concourse.bacc as bacc
import concourse.mybir as mybir
import concourse.tile as tile

F32 = mybir.dt.float32
F16 = mybir.dt.float16
ALPHA = 0.2
POISON = 244.0  # ent=-244, wr=+244 -> We=-59536 (fp16-exact), exp -> 0

N, K, D = 40000, 32, 100
N_CORES = 8
P = 128            # rows per tile == SBUF partitions
J = 2              # tiles per pair
STORE_CHUNK = 8    # tiles per output store
_N_TILES_FULL = 40  # 8 cores * 40 tiles * 128 rows = 40960 >= 40000

import os as _os
DVE_E_K = int(_os.environ.get("GAT_DVE_E_K", "9"))   # k's whose e-sum runs as DVE tree; rest ACT
GPS_E_L1 = int(_os.environ.get("GAT_GPS_E_L1", "1"))  # e-tree level-1 on GPSIMD
GPS_H_L1 = int(_os.environ.get("GAT_GPS_H_L1", "1"))  # h-tree level-1 on GPSIMD
GPS_EPI = int(_os.environ.get("GAT_GPS_EPI", "0"))    # epilogue: GPSIMD can't read PSUM


def _tree_steps(s):
    """Halving steps for an in-place prefix tree-sum of s elements:
    out[0:h] += in[keep:s], leaving keep = s - h live. Ends at s == 2."""
    steps = []
    while s > 2:
        h = s // 2
        steps.append((h, s - h, s))
        s = s - h
    return steps


def build(n_tiles: int, repeats: int = 1, mode: str = "full", klist=None):
    if klist is None:
        klist = [K] * (n_tiles // J)
    assert len(klist) == n_tiles // J
    sumkf = sum(2 * J * kp * D for kp in klist)  # ent+wr combined elems/row-p

    nc = bacc.Bacc("TRN2", target_bir_lowering=False, debug=False,
                   num_devices=N_CORES)

    cw_d = nc.dram_tensor("cw", [P * sumkf], F16, kind="ExternalInput")
    itemb_d = nc.dram_tensor("itemb", [P, n_tiles * D], F16, kind="ExternalInput")
    wt_d = nc.dram_tensor("wt", [D, D], F16, kind="ExternalInput")   # W_out.T
    ident_d = nc.dram_tensor("ident", [P, P], F16, kind="ExternalInput")
    out_d = nc.dram_tensor("out", [P, n_tiles * D], F16, kind="ExternalOutput")

    kmax = max(klist)

    with tile.TileContext(nc) as tc, ExitStack() as ctx:
        const = ctx.enter_context(tc.tile_pool(name="const", bufs=1))
        cwp = ctx.enter_context(tc.tile_pool(name="cwp", bufs=3))
        wep = ctx.enter_context(tc.tile_pool(name="wep", bufs=2))
        qp = ctx.enter_context(tc.tile_pool(name="qp", bufs=2))
        esp = ctx.enter_context(tc.tile_pool(name="esp", bufs=2))
        small = ctx.enter_context(tc.tile_pool(name="small", bufs=2))
        psum = ctx.enter_context(tc.tile_pool(name="psum", bufs=2, space="PSUM"))

        itemb = const.tile([P, n_tiles * D], F16)
        wt = const.tile([D, D], F16)
        ident = const.tile([P, P], F16)
        out_all = const.tile([P, n_tiles * D], F16)
        nc.sync.dma_start(itemb[:], itemb_d[:])
        nc.sync.dma_start(wt[:], wt_d[:])
        nc.sync.dma_start(ident[:], ident_d[:])

        def tile_loop():
            body_pairs(nc, n_tiles, klist, kmax, cw_d, out_d, itemb, wt,
                       ident, out_all, cwp, wep, qp, esp, small, psum, mode)

        if repeats > 1:
            with tc.For_i(0, repeats, 1):
                tile_loop()
        else:
            tile_loop()

    nc.compile()
    return nc


def body_pairs(nc, n_tiles, klist, kmax, cw_d, out_d, itemb, wt, ident,
               out_all, cwp, wep, qp, esp, small, psum, mode):
    AF = mybir.ActivationFunctionType
    AL = mybir.AluOpType

    off = 0    # element offset into the packed combined buffer
    for pg in range(n_tiles // J):
        kp = klist[pg]
        kf = kp * D
        blk = 2 * J * P * kf

        # one interleaved DMA: [:, :J*kf] = ent, [:, J*kf:] = wr (k-innermost)
        cw = cwp.tile([P, 2 * J * kmax * D], F16, tag="cw")
        nc.sync.dma_start(
            cw[:, :2 * J * kf].rearrange("p (t j f) -> p t j f", t=2, j=J),
            cw_d[off:off + blk].rearrange("(t j p f) -> p t j f",
                                          t=2, j=J, p=P))
        ent = cw[:, :J * kf]
        wr = cw[:, J * kf:2 * J * kf]

        if mode == "dma":
            for j in range(J):
                t = pg * J + j
                nc.vector.tensor_copy(out_all[:, t * D:(t + 1) * D],
                                      ent[:, j * kf:j * kf + D])
            if (pg + 1) % (STORE_CHUNK // J) == 0:
                csl = slice((pg + 1 - STORE_CHUNK // J) * J * D,
                            (pg + 1) * J * D)
                nc.sync.dma_start(out_d[:, csl], out_all[:, csl])
            off += blk
            continue

        # We = ent * wr   (DVE, fp16 2x mode)
        we = wep.tile([P, J * kmax * D], F16, tag="we")
        nc.vector.tensor_mul(we[:, :J * kf], ent, wr)
        we4 = we[:, :J * kf].rearrange("p (j d k) -> p j d k", j=J, k=kp)

        # ---- e_{j,k} = sum_d We[j, d, k] ----
        e = small.tile([P, J * kmax], F32, tag="e")
        e3 = e[:, :J * kp].rearrange("p (j k) -> p j k", j=J)
        kappa = min(kp, DVE_E_K)
        # first kappa k's: fp16 tree over d (DVE; level 1 optionally GPSIMD)
        es = esp.tile([P, J * 50 * kmax], F16, tag="es")
        es4 = es[:, :J * 50 * kappa].rearrange("p (j d k) -> p j d k",
                                               j=J, d=50)
        eng1 = nc.gpsimd if GPS_E_L1 else nc.vector
        eng1.tensor_add(es4, we4[:, :, 0:50, 0:kappa],
                        we4[:, :, 50:100, 0:kappa])
        for h, keep, s in _tree_steps(50):
            nc.vector.tensor_add(es4[:, :, 0:h, :], es4[:, :, 0:h, :],
                                 es4[:, :, keep:s, :])
        nc.vector.tensor_add(e3[:, :, 0:kappa].unsqueeze(2),
                             es4[:, :, 0:1, :], es4[:, :, 1:2, :])
        # remaining k's: ACT accumulate (sum_d of a strided [P, D] slice)
        if kp > kappa:
            trash = small.tile([P, D], F16, tag="trash")
            for j in range(J):
                wej = we[:, j * kf:(j + 1) * kf].rearrange(
                    "p (d k) -> p d k", k=kp)
                for k in range(kappa, kp):
                    nc.scalar.activation(
                        trash[:].unsqueeze(-1), wej[:, :, k:k + 1], AF.Copy,
                        accum_out=e[:, j * kp + k:j * kp + k + 1])

        # leaky relu (DVE): elr = max(alpha*e, e)
        elr = small.tile([P, J * kmax], F32, tag="elr")
        nc.vector.scalar_tensor_tensor(elr[:, :J * kp], e[:, :J * kp],
                                       ALPHA, e[:, :J * kp],
                                       op0=AL.mult, op1=AL.max)
        # exp + per-j sum (ACT, fused accumulate)
        p = small.tile([P, J * kmax], F32, tag="p")
        sumexp = small.tile([P, J], F32, tag="sumexp")
        for j in range(J):
            jsl = slice(j * kp, (j + 1) * kp)
            nc.scalar.activation(p[:, jsl], elr[:, jsl], AF.Exp,
                                 accum_out=sumexp[:, j:j + 1])
        rs = small.tile([P, J], F32, tag="rs")
        nc.vector.reciprocal(rs[:], sumexp[:])
        # normalized attention, fp16 (safe: in [0,1])
        ph = small.tile([P, J * kmax], F16, tag="ph")
        for j in range(J):
            jsl = slice(j * kp, (j + 1) * kp)
            nc.vector.scalar_tensor_tensor(ph[:, jsl], p[:, jsl],
                                           rs[:, j:j + 1], p[:, jsl],
                                           op0=AL.mult, op1=AL.bypass)

        # q = We * a  (DVE fp16 2x: broadcast over d keeps k innermost)
        q = qp.tile([P, J * kmax * D], F16, tag="q")
        q4 = q[:, :J * kf].rearrange("p (j d k) -> p j d k", j=J, k=kp)
        ph4 = (ph[:, :J * kp].rearrange("p (j k) -> p j k", j=J)
               .unsqueeze(2).broadcast_to([P, J, D, kp]))
        nc.vector.tensor_mul(q4, we4, ph4)

        # h'[j, d] = sum_k q: fp16 tree over k (level 1 optionally GPSIMD)
        hsteps = _tree_steps(kp)
        hu = small.tile([P, J * D], F16, tag="hu")
        for i, (h, keep, s) in enumerate(hsteps):
            eng = nc.gpsimd if (GPS_H_L1 and i == 0) else nc.vector
            eng.tensor_add(q4[:, :, :, 0:h], q4[:, :, :, 0:h],
                           q4[:, :, :, keep:s])
        nc.vector.tensor_add(
            hu[:].rearrange("p (j d) -> p j d", j=J).unsqueeze(-1),
            q4[:, :, :, 0:1], q4[:, :, :, 1:2])

        for j in range(J):
            t = pg * J + j
            # transpose h' -> [D, P] (PE fp16), copy PSUM->SBUF (ACT)
            ht_ps = psum.tile([D, P], F16, tag="htp")
            nc.tensor.transpose(ht_ps[:], hu[:, j * D:(j + 1) * D], ident[:])
            ht = small.tile([D, P], F16, tag="ht")
            nc.scalar.copy(ht[:], ht_ps[:])
            # x = h' @ W_out.T  (PE fp16)
            x_ps = psum.tile([P, D], F32, tag="xps")
            nc.tensor.matmul(x_ps[:], ht[:], wt[:], start=True, stop=True)
            # out = x + (item + b)  (residual epilogue)
            epi = nc.gpsimd if GPS_EPI else nc.vector
            epi.tensor_add(out_all[:, t * D:(t + 1) * D], x_ps[:],
                           itemb[:, t * D:(t + 1) * D])

        if (pg + 1) % (STORE_CHUNK // J) == 0:
            csl = slice((pg + 1 - STORE_CHUNK // J) * J * D,
                        (pg + 1) * J * D)
            nc.sync.dma_start(out_d[:, csl], out_all[:, csl])

        off += blk

    n_pairs = n_tiles // J
    rem = n_pairs % (STORE_CHUNK // J)
    if rem:
        csl = slice((n_pairs - rem) * J * D, n_pairs * J * D)
        nc.sync.dma_start(out_d[:, csl], out_all[:, csl])


def _shard_host(item_embs, entity_embs, w_r, adj, W_out, b_out, n_tiles):
    """Sort rows by active-neighbor count, pack active k's first, poison the
    masked tail slots, transpose each row to [D, kp] (k innermost), fp16,
    and interleave ent|wr into one per-core buffer. Pairs striped across
    cores as v1. Returns (in_maps, klist, order)."""
    rows = n_tiles * P
    n_pad = N_CORES * rows
    n_pairs = n_tiles // J

    ent = np.asarray(entity_embs, np.float32).reshape(N, K, D)
    wr = np.asarray(w_r, np.float32).reshape(N, K, D)
    adjf = np.asarray(adj).astype(np.float32)
    itemb = np.asarray(item_embs, np.float32) + np.asarray(b_out, np.float32)

    pad = n_pad - N
    ent = np.pad(ent, ((0, pad), (0, 0), (0, 0)))
    wr = np.pad(wr, ((0, pad), (0, 0), (0, 0)))
    # padding rows: one active zero neighbor -> e=0, sumexp=1 (count 1
    # sorts them to the sparse end); their output rows are discarded.
    adjp = np.pad(adjf, ((0, pad), (0, 0)))
    adjp[N:, 0] = 1.0
    itemb = np.pad(itemb, ((0, pad), (0, 0)))

    counts = adjp.sum(1).astype(np.int64)
    order = np.argsort(counts, kind="stable")

    pair_k = counts[order].reshape(-1, J * P).max(1)
    klist = [max(2, int(pair_k[8 * j: 8 * j + 8].max()))
             for j in range(n_pairs)]

    ai_full = np.argsort(1.0 - adjp, axis=1, kind="stable")  # active first

    wt = np.ascontiguousarray(np.asarray(W_out, np.float32).T).astype(np.float16)
    ident = np.eye(P, dtype=np.float16)

    in_maps = []
    for c in range(N_CORES):
        cw_parts = []
        it_sw = np.empty((P, n_tiles * D), np.float16)
        for j in range(n_pairs):
            g = 8 * j + c
            rsel = order[g * J * P:(g + 1) * J * P]
            kp = klist[j]
            ai = ai_full[rsel, :kp]
            cnt = counts[rsel]                               # [256]
            eg = np.take_along_axis(ent[rsel], ai[:, :, None], 1)  # [256,kp,D]
            wg = np.take_along_axis(wr[rsel], ai[:, :, None], 1)
            # poison masked tail slots: We[d0] = -POISON^2, rest 0 -> exp=0
            mask = np.arange(kp)[None, :] >= cnt[:, None]    # [256, kp]
            eg[mask] = 0.0
            wg[mask] = 0.0
            eg[:, :, 0][mask] = -POISON
            wg[:, :, 0][mask] = POISON
            # k-innermost: [256, kp, D] -> [256, D, kp]; fp16
            eg = eg.transpose(0, 2, 1).astype(np.float16)
            wg = wg.transpose(0, 2, 1).astype(np.float16)
            cw_parts.append(eg.ravel())
            cw_parts.append(wg.ravel())
            it = itemb[rsel].reshape(J, P, D).transpose(1, 0, 2)
            it_sw[:, j * J * D:(j + 1) * J * D] = \
                it.reshape(P, J * D).astype(np.float16)
        in_maps.append({
            "cw": np.concatenate(cw_parts),
            "itemb": it_sw,
            "wt": wt,
            "ident": ident,
        })
    return in_maps, klist, order


def _unshard_host(results, n_tiles, order):
    n_pairs = n_tiles // J
    res_sorted = np.empty((N_CORES * n_tiles * P, D), np.float32)
    for c in range(N_CORES):
        o = results[c]["out"].astype(np.float32)  # [P, n_tiles * D] fp16
        for j in range(n_pairs):
            g = 8 * j + c
            blk = (o[:, j * J * D:(j + 1) * J * D]
                   .reshape(P, J, D).transpose(1, 0, 2).reshape(J * P, D))
            res_sorted[g * J * P:(g + 1) * J * P] = blk
    out = np.empty_like(res_sorted)
    out[order] = res_sorted
    return out[:N]


def kernel(item_embs, entity_embs, w_r, adj, W_out, b_out):
    from concourse.bass_utils import run_bass_kernel_spmd

    in_maps, klist, order = _shard_host(item_embs, entity_embs, w_r, adj,
                                        W_out, b_out, _N_TILES_FULL)
    nc = build(_N_TILES_FULL, klist=klist)
    res = run_bass_kernel_spmd(nc, in_maps, core_ids=list(range(N_CORES)))
    return _unshard_host(res.results, _N_TILES_FULL, order).astype(np.float32)


# revision 10
# speedup vs baseline: 1.5654x; 1.5654x over previous
"""Trainium2 Bass kernel for nn_GAT_39427799777563 (GAT message passing).

Math (per item row n, K=32 neighbors, D=100 dims):
    We   = entity_embs * w_r                  # [K, D] elementwise
    e_k  = sum_d We[k, d]                     # neighbor logits
    a_k  = softmax_k(leaky_relu(e_k)) masked by adj
    h'   = sum_k a_k * We[k, :]               # weighted neighbor sum
    x    = h' @ W_out.T + b_out + item_embs

v2 design (vs the fp32 v1 at ~307us):
  * fp16 everywhere on the wire: ent/wr are loaded as one interleaved
    fp16 buffer (halves HBM traffic, the roofline term). fp16 (not bf16)
    because exp() amplifies e-sum rounding ~10x: bf16 inputs alone give
    1.7e-2 absmax-rel (gate 2e-2); fp16 gives 7.3e-3 (simulated).
  * k-innermost layout [row, j, d, k]: every elementwise op and tree-add
    has a packed 2-byte innermost AP dim, which turns on the DVE 2x mode
    (tensor_tensor 2x_1p). Crucially the attention-broadcast multiply
    q = We * a (broadcast over d) keeps k innermost-contiguous, so it
    runs 2x too - impossible in d-innermost layout (stride-0 innermost).
  * reductions as fp16 tree-adds (tensor_reduce never gets the 2x mode;
    tensor_tensor does): e-sum over d and h'-sum over k each cost ~half
    a strided reduce, with fp32 final level.
  * attention normalized BEFORE weighting (a = p/sum(p) in [0,1], fp16-
    safe; raw exp(e) ~ 1e17 is not), so the matmul epilogue is a plain
    residual add.
  * adj mask folded into the host packing: masked/padding slots get a
    poison pair ent[d0] = -244, wr[d0] = 244 (product -59536, exp -> 0
    exactly in fp32), so no adj tensor is loaded and no mask multiply.
  * engine balance per 256-row pair (DMA floor ~4.8us): DVE does the two
    big 2x multiplies + most tree levels (~5us); ACT takes the e-sums of
    the last few k's via activation(Copy, accum_out) plus exp and the
    PSUM->SBUF copies (~4.5us); GPSIMD takes the h'-tree first level and
    the residual epilogue (~3.8us); PE does transpose + the 100x100
    linear in fp16.

Sparsity packing as v1: active k's packed front per row, rows sorted by
count, 256-row pairs striped across the 8 SPMD cores, per-pair-slot K =
max over its 8 cores. Rows un-permuted on host after the gather.
"""

from contextlib import ExitStack

import numpy as np

import concourse.bass as bass
import concourse.bacc as bacc
import concourse.mybir as mybir
import concourse.tile as tile

F32 = mybir.dt.float32
F16 = mybir.dt.float16
ALPHA = 0.2
POISON = 244.0  # ent=-244, wr=+244 -> We=-59536 (fp16-exact), exp -> 0

N, K, D = 40000, 32, 100
N_CORES = 8
P = 128            # rows per tile == SBUF partitions
J = 2              # tiles per pair
STORE_CHUNK = 8    # tiles per output store
_N_TILES_FULL = 40  # 8 cores * 40 tiles * 128 rows = 40960 >= 40000

import os as _os
# engine-balance knobs (fractions of the k axis handed to the helper engine)
Q_ACT = int(_os.environ.get("GAT_Q_ACT", "9"))        # k's of q-mul on ACT (Copy+scale)
E_L1_POOL = float(_os.environ.get("GAT_E_L1_POOL", "1.0"))  # frac of e-tree L1 on GPSIMD
H_L1_POOL = float(_os.environ.get("GAT_H_L1_POOL", "0.66"))  # frac of h-tree L1 on GPSIMD
BUFS = int(_os.environ.get("GAT_BUFS", "2"))          # compute pool double/triple buffering


def _tree_steps(s):
    """Halving steps for an in-place prefix tree-sum of s elements:
    out[0:h] += in[keep:s], leaving keep = s - h live. Ends at s == 2."""
    steps = []
    while s > 2:
        h = s // 2
        steps.append((h, s - h, s))
        s = s - h
    return steps


def build(n_tiles: int, repeats: int = 1, mode: str = "full", klist=None):
    if klist is None:
        klist = [K] * (n_tiles // J)
    assert len(klist) == n_tiles // J
    sumkf = sum(2 * J * kp * D for kp in klist)  # ent+wr combined elems/row-p

    nc = bacc.Bacc("TRN2", target_bir_lowering=False, debug=False,
                   num_devices=N_CORES)

    cw_d = nc.dram_tensor("cw", [P * sumkf], F16, kind="ExternalInput")
    itemb_d = nc.dram_tensor("itemb", [P, n_tiles * D], F16, kind="ExternalInput")
    wt_d = nc.dram_tensor("wt", [D, D], F16, kind="ExternalInput")   # W_out.T
    ident_d = nc.dram_tensor("ident", [P, P], F16, kind="ExternalInput")
    out_d = nc.dram_tensor("out", [P, n_tiles * D], F16, kind="ExternalOutput")

    kmax = max(klist)

    with tile.TileContext(nc) as tc, ExitStack() as ctx:
        const = ctx.enter_context(tc.tile_pool(name="const", bufs=1))
        cwp = ctx.enter_context(tc.tile_pool(name="cwp", bufs=BUFS + 1))
        wep = ctx.enter_context(tc.tile_pool(name="wep", bufs=BUFS))
        qp = ctx.enter_context(tc.tile_pool(name="qp", bufs=BUFS))
        esp = ctx.enter_context(tc.tile_pool(name="esp", bufs=BUFS))
        small = ctx.enter_context(tc.tile_pool(name="small", bufs=BUFS))
        psum = ctx.enter_context(tc.tile_pool(name="psum", bufs=2, space="PSUM"))

        itemb = const.tile([P, n_tiles * D], F16)
        wt = const.tile([D, D], F16)
        ident = const.tile([P, P], F16)
        out_all = const.tile([P, n_tiles * D], F16)
        nc.sync.dma_start(itemb[:], itemb_d[:])
        nc.sync.dma_start(wt[:], wt_d[:])
        nc.sync.dma_start(ident[:], ident_d[:])

        def tile_loop():
            body_pairs(nc, n_tiles, klist, kmax, cw_d, out_d, itemb, wt,
                       ident, out_all, cwp, wep, qp, esp, small, psum, mode)

        if repeats > 1:
            with tc.For_i(0, repeats, 1):
                tile_loop()
        else:
            tile_loop()

    nc.compile()
    return nc


def body_pairs(nc, n_tiles, klist, kmax, cw_d, out_d, itemb, wt, ident,
               out_all, cwp, wep, qp, esp, small, psum, mode):
    AF = mybir.ActivationFunctionType
    AL = mybir.AluOpType

    off = 0    # element offset into the packed combined buffer
    for pg in range(n_tiles // J):
        kp = klist[pg]
        kf = kp * D
        blk = 2 * J * P * kf

        # one interleaved DMA: [:, :J*kf] = ent, [:, J*kf:] = wr (k-innermost)
        cw = cwp.tile([P, 2 * J * kmax * D], F16, tag="cw")
        nc.sync.dma_start(
            cw[:, :2 * J * kf].rearrange("p (t j f) -> p t j f", t=2, j=J),
            cw_d[off:off + blk].rearrange("(t j p f) -> p t j f",
                                          t=2, j=J, p=P))
        ent = cw[:, :J * kf]
        wr = cw[:, J * kf:2 * J * kf]

        if mode == "dma":
            for j in range(J):
                t = pg * J + j
                nc.vector.tensor_copy(out_all[:, t * D:(t + 1) * D],
                                      ent[:, j * kf:j * kf + D])
            if (pg + 1) % (STORE_CHUNK // J) == 0:
                csl = slice((pg + 1 - STORE_CHUNK // J) * J * D,
                            (pg + 1) * J * D)
                nc.sync.dma_start(out_d[:, csl], out_all[:, csl])
            off += blk
            continue

        # We = ent * wr   (DVE, fp16 2x mode)
        we = wep.tile([P, J * kmax * D], F16, tag="we")
        nc.vector.tensor_mul(we[:, :J * kf], ent, wr)
        we4 = we[:, :J * kf].rearrange("p (j d k) -> p j d k", j=J, k=kp)

        # ---- e_{j,k} = sum_d We[j, d, k]: fp16 tree over d ----
        # level 1 (100 -> 50) split between GPSIMD and DVE along k; the
        # remaining levels run on DVE (final level emits fp32).
        e = small.tile([P, J * kmax], F32, tag="e")
        e3 = e[:, :J * kp].rearrange("p (j k) -> p j k", j=J)
        es = esp.tile([P, J * 50 * kmax], F16, tag="es")
        es4 = es[:, :J * 50 * kp].rearrange("p (j d k) -> p j d k",
                                            j=J, d=50)
        ep = round(kp * E_L1_POOL)
        if ep > 0:
            nc.gpsimd.tensor_add(es4[:, :, :, 0:ep],
                                 we4[:, :, 0:50, 0:ep],
                                 we4[:, :, 50:100, 0:ep])
        if ep < kp:
            nc.vector.tensor_add(es4[:, :, :, ep:kp],
                                 we4[:, :, 0:50, ep:kp],
                                 we4[:, :, 50:100, ep:kp])
        for h, keep, s in _tree_steps(50):
            nc.vector.tensor_add(es4[:, :, 0:h, :], es4[:, :, 0:h, :],
                                 es4[:, :, keep:s, :])
        nc.vector.tensor_add(e3.unsqueeze(2),
                             es4[:, :, 0:1, :], es4[:, :, 1:2, :])

        # leaky relu (DVE): elr = max(alpha*e, e)
        elr = small.tile([P, J * kmax], F32, tag="elr")
        nc.vector.scalar_tensor_tensor(elr[:, :J * kp], e[:, :J * kp],
                                       ALPHA, e[:, :J * kp],
                                       op0=AL.mult, op1=AL.max)
        # exp (ACT, no accumulator read), then per-j sum on DVE
        p = small.tile([P, J * kmax], F32, tag="p")
        nc.scalar.activation(p[:, :J * kp], elr[:, :J * kp], AF.Exp)
        sumexp = small.tile([P, J], F32, tag="sumexp")
        nc.vector.tensor_reduce(
            sumexp[:], p[:, :J * kp].rearrange("p (j k) -> p j k", j=J),
            axis=mybir.AxisListType.X, op=AL.add)
        rs = small.tile([P, J], F32, tag="rs")
        nc.vector.reciprocal(rs[:], sumexp[:])
        # normalized attention, fp16 (safe: in [0,1])
        ph = small.tile([P, J * kmax], F16, tag="ph")
        for j in range(J):
            jsl = slice(j * kp, (j + 1) * kp)
            nc.vector.scalar_tensor_tensor(ph[:, jsl], p[:, jsl],
                                           rs[:, j:j + 1], p[:, jsl],
                                           op0=AL.mult, op1=AL.bypass)

        # q = We * a: DVE fp16 2x for the first kq k's (broadcast over d
        # keeps k innermost); ACT Copy-with-scale for the rest.
        q = qp.tile([P, J * kmax * D], F16, tag="q")
        q4 = q[:, :J * kf].rearrange("p (j d k) -> p j d k", j=J, k=kp)
        kq = max(2, kp - Q_ACT)
        ph4 = (ph[:, :J * kp].rearrange("p (j k) -> p j k", j=J)
               [:, :, 0:kq].unsqueeze(2).broadcast_to([P, J, D, kq]))
        nc.vector.tensor_mul(q4[:, :, :, 0:kq], we4[:, :, :, 0:kq], ph4)
        for j in range(J):
            for k in range(kq, kp):
                nc.scalar.activation(q4[:, j:j + 1, :, k:k + 1],
                                     we4[:, j:j + 1, :, k:k + 1], AF.Copy,
                                     scale=ph[:, j * kp + k:j * kp + k + 1])

        # h'[j, d] = sum_k q: fp16 tree over k; level 1 split GPSIMD/DVE
        hu = small.tile([P, J * D], F16, tag="hu")
        first = True
        for h, keep, s in _tree_steps(kp):
            if first:
                hp = round(h * H_L1_POOL)
                if hp > 0:
                    nc.gpsimd.tensor_add(q4[:, :, :, 0:hp],
                                         q4[:, :, :, 0:hp],
                                         q4[:, :, :, keep:keep + hp])
                if hp < h:
                    nc.vector.tensor_add(q4[:, :, :, hp:h],
                                         q4[:, :, :, hp:h],
                                         q4[:, :, :, keep + hp:s])
                first = False
            else:
                nc.vector.tensor_add(q4[:, :, :, 0:h], q4[:, :, :, 0:h],
                                     q4[:, :, :, keep:s])
        nc.vector.tensor_add(
            hu[:].rearrange("p (j d) -> p j d", j=J).unsqueeze(-1),
            q4[:, :, :, 0:1], q4[:, :, :, 1:2])

        for j in range(J):
            t = pg * J + j
            # transpose h' -> [D, P] (PE fp16), copy PSUM->SBUF (ACT)
            ht_ps = psum.tile([D, P], F16, tag="htp")
            nc.tensor.transpose(ht_ps[:], hu[:, j * D:(j + 1) * D], ident[:])
            ht = small.tile([D, P], F16, tag="ht")
            nc.scalar.copy(ht[:], ht_ps[:])
            # x = h' @ W_out.T  (PE fp16)
            x_ps = psum.tile([P, D], F32, tag="xps")
            nc.tensor.matmul(x_ps[:], ht[:], wt[:], start=True, stop=True)
            # out = x + (item + b)  (residual epilogue; GPSIMD can't read PSUM)
            nc.vector.tensor_add(out_all[:, t * D:(t + 1) * D], x_ps[:],
                                 itemb[:, t * D:(t + 1) * D])

        if (pg + 1) % (STORE_CHUNK // J) == 0:
            csl = slice((pg + 1 - STORE_CHUNK // J) * J * D,
                        (pg + 1) * J * D)
            nc.sync.dma_start(out_d[:, csl], out_all[:, csl])

        off += blk

    n_pairs = n_tiles // J
    rem = n_pairs % (STORE_CHUNK // J)
    if rem:
        csl = slice((n_pairs - rem) * J * D, n_pairs * J * D)
        nc.sync.dma_start(out_d[:, csl], out_all[:, csl])


def _shard_host(item_embs, entity_embs, w_r, adj, W_out, b_out, n_tiles):
    """Sort rows by active-neighbor count, pack active k's first, poison the
    masked tail slots, transpose each row to [D, kp] (k innermost), fp16,
    and interleave ent|wr into one per-core buffer. Pairs striped across
    cores as v1. Returns (in_maps, klist, order)."""
    rows = n_tiles * P
    n_pad = N_CORES * rows
    n_pairs = n_tiles // J

    ent = np.asarray(entity_embs, np.float32).reshape(N, K, D)
    wr = np.asarray(w_r, np.float32).reshape(N, K, D)
    adjf = np.asarray(adj).astype(np.float32)
    itemb = np.asarray(item_embs, np.float32) + np.asarray(b_out, np.float32)

    pad = n_pad - N
    ent = np.pad(ent, ((0, pad), (0, 0), (0, 0)))
    wr = np.pad(wr, ((0, pad), (0, 0), (0, 0)))
    # padding rows: one active zero neighbor -> e=0, sumexp=1 (count 1
    # sorts them to the sparse end); their output rows are discarded.
    adjp = np.pad(adjf, ((0, pad), (0, 0)))
    adjp[N:, 0] = 1.0
    itemb = np.pad(itemb, ((0, pad), (0, 0)))

    counts = adjp.sum(1).astype(np.int64)
    order = np.argsort(counts, kind="stable")

    pair_k = counts[order].reshape(-1, J * P).max(1)
    klist = [max(2, int(pair_k[8 * j: 8 * j + 8].max()))
             for j in range(n_pairs)]

    ai_full = np.argsort(1.0 - adjp, axis=1, kind="stable")  # active first

    wt = np.ascontiguousarray(np.asarray(W_out, np.float32).T).astype(np.float16)
    ident = np.eye(P, dtype=np.float16)

    in_maps = []
    for c in range(N_CORES):
        cw_parts = []
        it_sw = np.empty((P, n_tiles * D), np.float16)
        for j in range(n_pairs):
            g = 8 * j + c
            rsel = order[g * J * P:(g + 1) * J * P]
            kp = klist[j]
            ai = ai_full[rsel, :kp]
            cnt = counts[rsel]                               # [256]
            eg = np.take_along_axis(ent[rsel], ai[:, :, None], 1)  # [256,kp,D]
            wg = np.take_along_axis(wr[rsel], ai[:, :, None], 1)
            # poison masked tail slots: We[d0] = -POISON^2, rest 0 -> exp=0
            mask = np.arange(kp)[None, :] >= cnt[:, None]    # [256, kp]
            eg[mask] = 0.0
            wg[mask] = 0.0
            eg[:, :, 0][mask] = -POISON
            wg[:, :, 0][mask] = POISON
            # k-innermost: [256, kp, D] -> [256, D, kp]; fp16
            eg = eg.transpose(0, 2, 1).astype(np.float16)
            wg = wg.transpose(0, 2, 1).astype(np.float16)
            cw_parts.append(eg.ravel())
            cw_parts.append(wg.ravel())
            it = itemb[rsel].reshape(J, P, D).transpose(1, 0, 2)
            it_sw[:, j * J * D:(j + 1) * J * D] = \
                it.reshape(P, J * D).astype(np.float16)
        in_maps.append({
            "cw": np.concatenate(cw_parts),
            "itemb": it_sw,
            "wt": wt,
            "ident": ident,
        })
    return in_maps, klist, order


def _unshard_host(results, n_tiles, order):
    n_pairs = n_tiles // J
    res_sorted = np.empty((N_CORES * n_tiles * P, D), np.float32)
    for c in range(N_CORES):
        o = results[c]["out"].astype(np.float32)  # [P, n_tiles * D] fp16
        for j in range(n_pairs):
            g = 8 * j + c
            blk = (o[:, j * J * D:(j + 1) * J * D]
                   .reshape(P, J, D).transpose(1, 0, 2).reshape(J * P, D))
            res_sorted[g * J * P:(g + 1) * J * P] = blk
    out = np.empty_like(res_sorted)
    out[order] = res_sorted
    return out[:N]


def kernel(item_embs, entity_embs, w_r, adj, W_out, b_out):
    from concourse.bass_utils import run_bass_kernel_spmd

    in_maps, klist, order = _shard_host(item_embs, entity_embs, w_r, adj,
                                        W_out, b_out, _N_TILES_FULL)
    nc = build(_N_TILES_FULL, klist=klist)
    res = run_bass_kernel_spmd(nc, in_maps, core_ids=list(range(N_CORES)))
    return _unshard_host(res.results, _N_TILES_FULL, order).astype(np.float32)
